# revision 1
# baseline (speedup 1.0000x reference)
"""Trainium2 Bass kernel for nn_Attention_41102837023186 (sparse GQA attention).

Head-tensor-parallel over 8 NeuronCores: core c owns q heads [3c, 3c+3) and
kv head c. Per core: rms-norm folded into weights/scales, QKV projections
(fp32r), RoPE+q/k-rms on DVE/ACT, block-sparse attention with the ragged-range
mask, chunked AllGather of the attention output, then the Wproj column block.

kernel(**inputs) takes the FULL unsharded inputs and returns the FULL output.
"""

import numpy as np

FULL_CFG = dict(S=3072, H=3072, HQ=24, HKV=8, D=128)
NCORES = 8
SC = 512  # token chunk (free-dim tile)
EPS = 1e-6
NEG = -1e30

_uid = [0]


# ---------------------------------------------------------------------------
# BIR post-fix: this walrus build accepts only ONE sem wait per instruction;
# Tile emits more (tail drain, DMA fan-ins). Split overflow waits onto
# preceding NoOp instructions on the same engine.
# ---------------------------------------------------------------------------
def _fix_bir_json_bytes(raw: bytes) -> bytes:
    import json as _json

    m = _json.loads(raw)
    changed = False
    for f in m.get("functions", []):
        for blk in f.get("blocks", []):
            out = []
            for inst in blk["instructions"]:
                si = inst.get("sync_info") or {}
                waits = si.get("on_wait") or []
                if len(waits) > 1:
                    changed = True
                    for w in waits[:-1]:
                        _uid[0] += 1
                        out.append(
                            {
                                "name": f"I-waitsplit-{_uid[0]}",
                                "opcode": "NoOp",
                                "engine": inst["engine"],
                                "ins": [],
                                "outs": [],
                                "debug": inst.get("debug", 0),
                                "sync_info": {"on_update": [], "on_wait": [w]},
                            }
                        )
                    si = dict(si)
                    si["on_wait"] = waits[-1:]
                    inst = dict(inst)
                    inst["sync_info"] = si
                out.append(inst)
            blk["instructions"] = out
    if not changed:
        return raw
    return _json.dumps(m).encode()


def _patch_bass(nc):
    import types

    orig = nc.to_json_bytes

    def patched(self):
        return _fix_bir_json_bytes(orig())

    nc.to_json_bytes = types.MethodType(patched, nc)
    return nc


# ---------------------------------------------------------------------------
# Host-side prep: fold norm weights, transpose layouts, range -> tile map
# ---------------------------------------------------------------------------
def _host_prep(x, cos, sin, pre_norm_w, q_norm_w, k_norm_w, Wq, Wk, Wv, Wproj,
               q_ranges, k_ranges, cfg):
    S, H, HQ, HKV, D = cfg["S"], cfg["H"], cfg["HQ"], cfg["HKV"], cfg["D"]
    HALF = D // 2
    NHQ = HQ // NCORES
    HD = HQ * D
    f32 = np.float32

    x = np.asarray(x, f32)
    cos2 = np.asarray(cos, f32).reshape(S, HALF)
    sin2 = np.asarray(sin, f32).reshape(S, HALF)
    w1 = (np.asarray(pre_norm_w, f32) + 1.0)
    qw1 = (np.asarray(q_norm_w, f32) + 1.0)
    kw1 = (np.asarray(k_norm_w, f32) + 1.0)
    Wq = np.asarray(Wq, f32) * w1[None, :]
    Wk = np.asarray(Wk, f32) * w1[None, :]
    Wv = np.asarray(Wv, f32) * w1[None, :]
    Wproj = np.asarray(Wproj, f32)
    qr = np.asarray(q_ranges).astype(np.int64)
    kr = np.asarray(k_ranges).astype(np.int64)

    xT = np.ascontiguousarray(x.T).astype(np.float16)  # [H, S]

    # rope packs [D, S]: rows 0:HALF scale for x_lo terms, HALF:D for x_hi
    def pack(tab, wvec):
        return np.ascontiguousarray(
            np.concatenate([tab.T * wvec[:HALF, None], tab.T * wvec[HALF:, None]],
                           axis=0)).astype(f32)

    cospack_q, sinpack_q = pack(cos2, qw1), pack(sin2, qw1)
    cospack_k, sinpack_k = pack(cos2, kw1), pack(sin2, kw1)

    # ragged-range tile map in scores^T orientation: allowed[k, q]
    allowed = np.zeros((S, S), dtype=bool)
    covered = np.zeros((S,), dtype=bool)
    for r in range(qr.shape[0]):
        q0, q1 = int(qr[r, 0]), int(qr[r, 1])
        k0, k1 = int(kr[r, 0]), int(kr[r, 1])
        q0, q1 = max(q0, 0), min(q1, S)
        k0, k1 = max(k0, 0), min(k1, S)
        if q1 > q0:
            covered[q0:q1] = True
            if k1 > k0:
                allowed[k0:k1, q0:q1] = True

    n_kt = S // D
    n_sc = S // SC
    masks = []
    chunk_plan = []  # per sc: list of (kt, mask_id_or_None)
    uncov_needed = []
    for sc in range(n_sc):
        plan = []
        qs = slice(sc * SC, (sc + 1) * SC)
        for kt in range(n_kt):
            sub = allowed[kt * D:(kt + 1) * D, qs]
            if sub.all():
                plan.append((kt, None))
            elif sub.any():
                masks.append(np.where(sub, np.float32(0), np.float32(NEG)))
                plan.append((kt, len(masks) - 1))
        chunk_plan.append(plan)
        cov_chunk = covered[qs]
        # den += 1 where this chunk's q has no allowed keys (avoid 0*inf)
        has_keys = allowed[:, qs].any(axis=0)
        uncov_needed.append(None if has_keys.all()
                            else (~has_keys).astype(f32)[None, :])

    masks_arr = (np.ascontiguousarray(np.stack(masks)) if masks
                 else np.zeros((1, D, SC), f32))

    cov_arr = covered.astype(f32)[None, :]  # [1, S], for output zeroing

    per_core = []
    for c in range(NCORES):
        wkvq = np.ascontiguousarray(
            np.concatenate(
                [Wk[c * D:(c + 1) * D].T, Wv[c * D:(c + 1) * D].T,
                 Wq[c * NHQ * D:(c + 1) * NHQ * D].T], axis=1)).astype(np.float16)
        outc = H // NCORES
        wpt = np.ascontiguousarray(
            Wproj[c * outc:(c + 1) * outc].T).astype(np.float16)  # [HD, H//NCORES]
        per_core.append(dict(xT=xT, wkvq=wkvq, wpt=wpt,
                             cospack_q=cospack_q, sinpack_q=sinpack_q,
                             cospack_k=cospack_k, sinpack_k=sinpack_k,
                             masks=masks_arr))
    spec = dict(chunk_plan=chunk_plan, uncov=uncov_needed, covered=cov_arr,
                all_covered=bool(covered.all()))
    return per_core, spec


# ---------------------------------------------------------------------------
# Device program (identical on all cores; SPMD over inputs)
# ---------------------------------------------------------------------------
def _build_program(cfg, spec, n_masks, same_packs):
    import concourse.bass as bass
    import concourse.tile as tile
    from concourse import mybir

    f32 = mybir.dt.float32
    f32r = mybir.dt.float32r
    f16 = mybir.dt.float16
    AF = mybir.ActivationFunctionType

    S, H, HQ, HKV, D = cfg["S"], cfg["H"], cfg["HQ"], cfg["HKV"], cfg["D"]
    HALF = D // 2
    NHQ = HQ // NCORES
    HD = HQ * D
    n_ht = H // D
    n_kt = S // D
    n_sc = S // SC
    n_st = SC // D  # 128-token subtiles per chunk
    OUTC = H // NCORES  # output columns per core
    QKSCALE = float(1.0 / np.sqrt(D))
    chunk_plan = spec["chunk_plan"]
    uncov = spec["uncov"]

    # which k-chunk must be complete before attention(sc) can run
    maxk = []
    for sc in range(n_sc):
        plan = chunk_plan[sc]
        maxk.append(((max(kt for kt, _ in plan) * D) // SC) if plan else 0)

    nc = bass.Bass(num_devices=NCORES)

    # register EPS as a const AP so activation(bias=EPS) can resolve it
    _epst = nc.alloc_sbuf_tensor("const-float32-eps", [128, 1], f32)
    nc.gpsimd.memset(_epst.ap(), EPS)
    nc.const_aps.aps[(f32, EPS)] = _epst.ap()
    _negt = nc.alloc_sbuf_tensor("const-float32-neg1", [128, 1], f32)
    nc.gpsimd.memset(_negt.ap(), -1.0)
    nc.const_aps.aps[(f32, -1.0)] = _negt.ap()
    nc.all_engine_barrier()

    xT_d = nc.dram_tensor("xT", [H, S], f16, kind="ExternalInput")
    wkvq_d = nc.dram_tensor("wkvq", [H, (2 + NHQ) * D], f16, kind="ExternalInput")
    wpt_d = nc.dram_tensor("wpt", [HD, OUTC], f16, kind="ExternalInput")
    cq_d = nc.dram_tensor("cospack_q", [D, S], f32, kind="ExternalInput")
    sq_d = nc.dram_tensor("sinpack_q", [D, S], f32, kind="ExternalInput")
    ck_d = nc.dram_tensor("cospack_k", [D, S], f32, kind="ExternalInput")
    sk_d = nc.dram_tensor("sinpack_k", [D, S], f32, kind="ExternalInput")
    masks_d = nc.dram_tensor("masks", [n_masks, D, SC], f32, kind="ExternalInput")
    out_d = nc.dram_tensor("out", [OUTC, S], f32, kind="ExternalOutput")

    r_dram = nc.dram_tensor("r_scratch", [1, S], f32)
    sync_in = nc.dram_tensor("sync_in", [1, 128], f32)
    sync_out = nc.dram_tensor("sync_out", [NCORES, 128], f32, addr_space="Shared")
    ag_in = [nc.dram_tensor(f"ag_in_{j}", [NHQ * D, SC], f16) for j in range(n_sc)]
    ag_out = [nc.dram_tensor(f"ag_out_{j}", [HD, SC], f16, addr_space="Shared")
              for j in range(n_sc)]

    uncov_d = None
    if any(u is not None for u in uncov):
        uncov_d = nc.dram_tensor("uncov", [1, S], f32, kind="ExternalInput")

    ident_d = nc.inline_tensor(np.eye(D, dtype=np.float32), name="ident128")
    ones_d = nc.inline_tensor(np.ones((D, 1), dtype=np.float32), name="ones128")
    onesr_d = nc.inline_tensor(np.ones((1, D), dtype=np.float32), name="ones1x128")

    from contextlib import ExitStack
    with tile.TileContext(nc) as tc, ExitStack() as ctx:
        pool = lambda *a, **k: ctx.enter_context(tc.tile_pool(*a, **k))
        const_p = pool(name="const", bufs=1)
        w_p = pool(name="wkvq", bufs=n_ht)
        wpt_p = pool(name="wpt", bufs=HD // D)
        big_p = pool(name="big", bufs=1)
        x_p = pool(name="x", bufs=4)
        sq_p = pool(name="sq", bufs=3)
        trig_p = pool(name="trig", bufs=2)
        rope_p = pool(name="rope", bufs=4)
        tmp_p = pool(name="tmp", bufs=2)
        qh_p = pool(name="qh", bufs=4)
        pexp_p = pool(name="pexp", bufs=4)
        row_p = pool(name="row", bufs=3)
        rb_p = pool(name="rb", bufs=2)
        at_p = pool(name="at", bufs=2)
        lt_p = pool(name="lt", bufs=6)
        os_p = pool(name="os", bufs=2)
        any_masks = any(mid is not None for plan in chunk_plan for _, mid in plan)
        mask_p = pool(name="mask", bufs=1) if any_masks else None
        psN = pool(name="psN", bufs=8, space="PSUM")

        ident = const_p.tile([D, D], f32r)
        nc.sync.dma_start(ident[:], ident_d.ap().bitcast(f32r))
        ones = const_p.tile([D, 1], f32r)
        nc.sync.dma_start(ones[:], ones_d.ap().bitcast(f32r))
        onesr = const_p.tile([1, D], f32r)
        nc.sync.dma_start(onesr[:], onesr_d.ap().bitcast(f32r))
        ones16 = const_p.tile([D, 1], f16)
        nc.vector.tensor_copy(ones16[:], ones[:].bitcast(f32))

        wkvq_sb = []
        for t in range(n_ht):
            w = w_p.tile([D, (2 + NHQ) * D], f16, tag="w")
            nc.sync.dma_start(w[:], wkvq_d[t * D:(t + 1) * D, :])
            wkvq_sb.append(w)
        wpt_sb = []

        def load_wpt():
            for t in range(HD // D):
                w = wpt_p.tile([D, OUTC], f16, tag="wp", name=f"wpt{t}")
                nc.sync.dma_start(w[:], wpt_d[t * D:(t + 1) * D, :])
                wpt_sb.append(w)

        # tiny collective up-front: absorbs inter-core dispatch skew while
        # weights stream in, so the first real AllGather isn't a barrier
        nc.gpsimd.collective_compute(
            "AllGather", mybir.AluOpType.bypass,
            replica_groups=[list(range(NCORES))],
            ins=[sync_in.ap()], outs=[sync_out.ap()],
        )

        khatT = big_p.tile([D, S], f16, tag="khat")   # [d, token]
        v_sb = big_p.tile([D, S], f16, tag="v")       # [token(kt-major), d]
        r_pp = big_p.tile([D, n_kt], f32, tag="rpp")   # per-partition r

        uncov_sb = None
        if uncov_d is not None:
            uncov_sb = big_p.tile([1, S], f32, tag="uncov")
            nc.sync.dma_start(uncov_sb[:], uncov_d[:, :])

        def rope_block(psrc, cos_t, sin_t, dst_ap, scale_sb):
            t1 = rope_p.tile([HALF, SC], f32, tag="rp")
            t2 = rope_p.tile([HALF, SC], f32, tag="rp")
            t3 = rope_p.tile([HALF, SC], f32, tag="rp")
            t4 = rope_p.tile([HALF, SC], f32, tag="rp")
            nc.vector.tensor_mul(t1[:], psrc[0:HALF, :], cos_t[0:HALF, :])
            nc.vector.tensor_mul(t2[:], psrc[HALF:D, :], sin_t[HALF:D, :])
            nc.vector.tensor_mul(t3[:], psrc[HALF:D, :], cos_t[HALF:D, :])
            nc.vector.tensor_mul(t4[:], psrc[0:HALF, :], sin_t[0:HALF, :])
            tmp = tmp_p.tile([D, SC], f32, tag="ropetmp")
            nc.vector.tensor_sub(tmp[0:HALF, :], t1[:], t2[:])
            nc.vector.tensor_add(tmp[HALF:D, :], t3[:], t4[:])
            nc.vector.tensor_mul(dst_ap, tmp[:], scale_sb)

        def rms_scale(p_raw):
            sq = sq_p.tile([D, SC], f32r, tag="sq", bufs=2)
            nc.scalar.activation(sq[:], p_raw[:], AF.Square)
            pss = psN.tile([1, SC], f32, tag="b")
            nc.tensor.matmul(pss[:], ones[:], sq[:], start=True, stop=True)
            tvar = row_p.tile([1, SC], f32, tag="row")
            nc.scalar.activation(tvar[:], pss[:], AF.Ln, scale=1.0 / D, bias=EPS)
            rq = row_p.tile([1, SC], f32r, tag="rowr")
            nc.scalar.activation(rq[:], tvar[:], AF.Exp, scale=-0.5)
            prb = psN.tile([D, SC], f32, tag="b")
            nc.tensor.matmul(prb[:], onesr[:], rq[:], start=True, stop=True)
            rb = rb_p.tile([D, SC], f32, tag="rb")
            nc.scalar.copy(rb[:], prb[:])
            return rb

        n_ct = OUTC // D

        def emit_proj(psc):
            pssl = slice(psc * SC, (psc + 1) * SC)
            po = [psN.tile([D, SC], f32, tag="b", name=f"po{_j}")
                  for _j in range(n_ct)]
            for t in range(HD // D):
                lt = lt_p.tile([D, SC], f16, tag="lt")
                nc.gpsimd.dma_start(lt[:],
                                    ag_out[psc][t * D:(t + 1) * D, :])
                for j in range(n_ct):
                    nc.tensor.matmul(po[j][:], wpt_sb[t][:, j * D:(j + 1) * D],
                                     lt[:], start=(t == 0),
                                     stop=(t == HD // D - 1))
            for j in range(n_ct):
                ob = os_p.tile([D, SC], f32, tag="os")
                nc.scalar.copy(ob[:], po[j][:])
                nc.sync.dma_start(out_d[j * D:(j + 1) * D, pssl], ob[:])

        def emit_attention_head(sc, h):
            ssl = slice(sc * SC, (sc + 1) * SC)
            plan = chunk_plan[sc]
            pattn = psN.tile([D, SC], f32, tag="b")
            pden = psN.tile([1, SC], f32, tag="b")
            pes = {}

            def emit_qk(i):
                kt, mid = plan[i]
                ps = psN.tile([D, SC], f32, tag="b")
                nc.tensor.matmul(ps[:], khatT[:, kt * D:(kt + 1) * D],
                                 qhat_all[sc][h][:], start=True, stop=True)
                if mid is not None:
                    mt = mask_p.tile([D, SC], f32, tag="m")
                    nc.sync.dma_start(mt[:], masks_d[mid, :, :])
                    nc.vector.tensor_add(ps[:], ps[:], mt[:])
                pe = pexp_p.tile([D, SC], f16, tag="pe")
                nc.scalar.activation(pe[:], ps[:], AF.Exp, scale=QKSCALE,
                                     bias=-1.0)
                pes[i] = pe

            def emit_pv(i):
                kt, mid = plan[i]
                pe = pes.pop(i)
                first, last = i == 0, i == len(plan) - 1
                nc.tensor.matmul(pattn[:], v_sb[:, kt * D:(kt + 1) * D],
                                 pe[:], start=first, stop=last)
                nc.tensor.matmul(pden[:], ones16[:], pe[:], start=first,
                                 stop=last)

            LAG = 3
            for i in range(len(plan)):
                emit_qk(i)
                if i >= LAG:
                    emit_pv(i - LAG)
            for i in range(max(0, len(plan) - LAG), len(plan)):
                emit_pv(i)

            at = at_p.tile([D, SC], f16, tag="at")
            if not plan:
                nc.vector.memset(at[:], 0.0)
            else:
                if uncov[sc] is not None:
                    nc.vector.tensor_add(pden[:], pden[:], uncov_sb[0:1, ssl])
                dln = row_p.tile([1, SC], f32, tag="row")
                nc.scalar.activation(dln[:], pden[:], AF.Ln)
                rec = row_p.tile([1, SC], f32r, tag="rowr")
                nc.scalar.activation(rec[:], dln[:], AF.Exp, scale=-1.0)
                prb = psN.tile([D, SC], f32, tag="b")
                nc.tensor.matmul(prb[:], onesr[:], rec[:], start=True, stop=True)
                rb2 = rb_p.tile([D, SC], f32, tag="rb")
                nc.vector.tensor_copy(rb2[:], prb[:])
                nc.vector.tensor_mul(at[:], pattn[:], rb2[:])
            nc.sync.dma_start(ag_in[sc][h * D:(h + 1) * D, :], at[:])

        # ---------------- fused main loop over token chunks ----------------
        qhat_all = {}
        proj_queue = []
        attn_done = 0
        load_wpt()
        for sc in range(n_sc):
            ssl = slice(sc * SC, (sc + 1) * SC)
            pk = psN.tile([D, SC], f32, tag="b")
            pv = psN.tile([D, SC], f32, tag="b")
            pss = psN.tile([1, SC], f32, tag="b")
            pq = [psN.tile([D, SC], f32, tag="b", name=f"pq{_h}")
                  for _h in range(NHQ)]
            for ht in range(n_ht):
                xt = x_p.tile([D, SC], f16, tag="x")
                nc.sync.dma_start(xt[:], xT_d[ht * D:(ht + 1) * D, ssl])
                st, sp = ht == 0, ht == n_ht - 1
                nc.tensor.matmul(pk[:], wkvq_sb[ht][:, 0:D], xt[:], start=st, stop=sp)
                nc.tensor.matmul(pv[:], wkvq_sb[ht][:, D:2 * D], xt[:], start=st, stop=sp)
                for h in range(NHQ):
                    nc.tensor.matmul(pq[h][:], wkvq_sb[ht][:, (2 + h) * D:(3 + h) * D],
                                     xt[:], start=st, stop=sp)
                sqx = sq_p.tile([D, SC], f32r, tag="sqx")
                nc.vector.tensor_mul(sqx[:], xt[:], xt[:])
                nc.tensor.matmul(pss[:], ones[:], sqx[:], start=st, stop=sp)
            # r chunk
            tvar = row_p.tile([1, SC], f32, tag="row")
            nc.scalar.activation(tvar[:], pss[:], AF.Ln, scale=1.0 / H, bias=EPS)
            r_chunk = row_p.tile([1, SC], f32, tag="row")
            nc.scalar.activation(r_chunk[:], tvar[:], AF.Exp, scale=-0.5)
            nc.sync.dma_start(r_dram[0:1, ssl], r_chunk[:])
            nc.sync.dma_start(
                r_pp[:, sc * n_st:(sc + 1) * n_st],
                r_dram[0:1, ssl].rearrange("o (j p) -> (o p) j", p=D))
            # k: rms + rope
            ckt = trig_p.tile([D, SC], f32, tag="ck")
            skt = trig_p.tile([D, SC], f32, tag="sk")
            nc.gpsimd.dma_start(ckt[:], ck_d[:, ssl])
            nc.gpsimd.dma_start(skt[:], sk_d[:, ssl])
            rb = rms_scale(pk)
            rope_block(pk, ckt, skt, khatT[:, ssl], rb[:])
            # v: copy then transpose to [token, d], scaled by r
            vt = tmp_p.tile([D, SC], f32r, tag="vt")
            nc.scalar.copy(vt[:], pv[:])
            for j in range(n_st):
                ptr = psN.tile([D, D], f32r, tag="b")
                nc.tensor.transpose(ptr[:], vt[:, j * D:(j + 1) * D], ident[:])
                kt = sc * n_st + j
                nc.scalar.activation(v_sb[:, kt * D:(kt + 1) * D],
                                     ptr[:].bitcast(f32), AF.Copy,
                                     scale=r_pp[:, kt:kt + 1])
            # q: rms + rope (reuse trig slots; reload when packs differ)
            if same_packs:
                cqt, sqt = ckt, skt
            else:
                cqt = trig_p.tile([D, SC], f32, tag="ck")
                sqt = trig_p.tile([D, SC], f32, tag="sk")
                nc.sync.dma_start(cqt[:], cq_d[:, ssl])
                nc.sync.dma_start(sqt[:], sq_d[:, ssl])
            qhat_all[sc] = []

            def emit_qrope(h):
                rbq = rms_scale(pq[h])
                qh = qh_p.tile([D, SC], f16, tag="qh")
                rope_block(pq[h], cqt, sqt, qh[:], rbq[:])
                qhat_all[sc].append(qh)

            emit_qrope(0)
            # fill the DVE-rope window with queued projection matmuls; keep
            # one chunk queued so the fill never waits on a just-fired AG
            while len(proj_queue) > 1:
                emit_proj(proj_queue.pop(0))
            # attention chunks whose key context is now available, with head
            # h's rope overlapping head h-1's attention
            ready = [sc2 for sc2 in range(n_sc) if maxk[sc2] == sc]
            for h in range(NHQ):
                if h + 1 < NHQ:
                    emit_qrope(h + 1)
                for sc2 in ready:
                    emit_attention_head(sc2, h)
            for sc2 in ready:
                nc.gpsimd.collective_compute(
                    "AllGather", mybir.AluOpType.bypass,
                    replica_groups=[list(range(NCORES))],
                    ins=[ag_in[sc2].ap()], outs=[ag_out[sc2].ap()],
                )
                proj_queue.append(sc2)
        while proj_queue:
            emit_proj(proj_queue.pop(0))

    return nc


def build_and_run(x, cos, sin, pre_norm_w, q_norm_w, k_norm_w, Wq, Wk, Wv,
                  Wproj, q_ranges, k_ranges, cfg=None, trace=False,
                  trace_kwargs=None):
    from concourse.bass_utils import run_bass_kernel_spmd

    cfg = cfg or FULL_CFG
    per_core, spec = _host_prep(x, cos, sin, pre_norm_w, q_norm_w, k_norm_w,
                                Wq, Wk, Wv, Wproj, q_ranges, k_ranges, cfg)
    n_masks = per_core[0]["masks"].shape[0]
    same_packs = (np.array_equal(per_core[0]["cospack_q"], per_core[0]["cospack_k"])
                  and np.array_equal(per_core[0]["sinpack_q"], per_core[0]["sinpack_k"]))
    nc = _build_program(cfg, spec, n_masks, same_packs)
    _patch_bass(nc)

    in_maps = []
    for c in range(NCORES):
        m = dict(per_core[c])
        if any(u is not None for u in spec["uncov"]):
            S = cfg["S"]
            ua = np.zeros((1, S), np.float32)
            for sc, u in enumerate(spec["uncov"]):
                if u is not None:
                    ua[0, sc * SC:(sc + 1) * SC] = u
            m["uncov"] = ua
        in_maps.append(m)

    kw = {}
    if trace:
        kw = dict(trace=True, trace_kwargs=trace_kwargs or {})
    res = run_bass_kernel_spmd(nc, in_maps, core_ids=list(range(NCORES)), **kw)
    out = np.concatenate([res.results[c]["out"] for c in range(NCORES)], axis=0).T
    if not spec["all_covered"]:
        out = out * spec["covered"].T  # zero uncovered rows
    return out, res


def kernel(**inputs):
    out, _ = build_and_run(**inputs)
    return out



# revision 12
# speedup vs baseline: 1.1060x; 1.1060x over previous
"""Trainium2 Bass kernel for nn_Attention_41102837023186 (sparse GQA attention).

Head-tensor-parallel over 8 NeuronCores: core c owns q heads [3c, 3c+3) and
kv head c. Structure: KV projections+rope for all token chunks first, then per
token chunk (largest-attention first) Q projection+rope+block-sparse attention
with column-windowed partial tiles, chunked AllGather of attention outputs,
and the Wproj column block overlapped with later chunks.

kernel(**inputs) takes the FULL unsharded inputs and returns the FULL output.
"""

import numpy as np

FULL_CFG = dict(S=3072, H=3072, HQ=24, HKV=8, D=128)
NCORES = 8
SC = 512  # token chunk (free-dim tile)
EPS = 1e-6
NEG = -1e30

_uid = [0]


# ---------------------------------------------------------------------------
# BIR post-fix: this walrus build accepts only ONE sem wait per instruction;
# Tile emits more (tail drain, DMA fan-ins). Split overflow waits onto
# preceding NoOp instructions on the same engine.
# ---------------------------------------------------------------------------
def _fix_bir_json_bytes(raw: bytes) -> bytes:
    import json as _json

    m = _json.loads(raw)
    changed = False
    for f in m.get("functions", []):
        for blk in f.get("blocks", []):
            out = []
            for inst in blk["instructions"]:
                si = inst.get("sync_info") or {}
                waits = si.get("on_wait") or []
                if len(waits) > 1:
                    changed = True
                    for w in waits[:-1]:
                        _uid[0] += 1
                        out.append(
                            {
                                "name": f"I-waitsplit-{_uid[0]}",
                                "opcode": "NoOp",
                                "engine": inst["engine"],
                                "ins": [],
                                "outs": [],
                                "debug": inst.get("debug", 0),
                                "sync_info": {"on_update": [], "on_wait": [w]},
                            }
                        )
                    si = dict(si)
                    si["on_wait"] = waits[-1:]
                    inst = dict(inst)
                    inst["sync_info"] = si
                out.append(inst)
            blk["instructions"] = out
    if not changed:
        return raw
    return _json.dumps(m).encode()


def _patch_bass(nc):
    import types

    orig = nc.to_json_bytes

    def patched(self):
        return _fix_bir_json_bytes(orig())

    nc.to_json_bytes = types.MethodType(patched, nc)
    return nc


# ---------------------------------------------------------------------------
# Host-side prep: fold norm weights, transpose layouts, range -> tile plan
# ---------------------------------------------------------------------------
def _host_prep(x, cos, sin, pre_norm_w, q_norm_w, k_norm_w, Wq, Wk, Wv, Wproj,
               q_ranges, k_ranges, cfg):
    S, H, HQ, HKV, D = cfg["S"], cfg["H"], cfg["HQ"], cfg["HKV"], cfg["D"]
    HALF = D // 2
    NHQ = HQ // NCORES
    f32 = np.float32

    x = np.asarray(x, f32)
    cos2 = np.asarray(cos, f32).reshape(S, HALF)
    sin2 = np.asarray(sin, f32).reshape(S, HALF)
    w1 = (np.asarray(pre_norm_w, f32) + 1.0)
    qw1 = (np.asarray(q_norm_w, f32) + 1.0)
    kw1 = (np.asarray(k_norm_w, f32) + 1.0)
    Wq = np.asarray(Wq, f32) * w1[None, :]
    Wk = np.asarray(Wk, f32) * w1[None, :]
    Wv = np.asarray(Wv, f32) * w1[None, :]
    Wproj = np.asarray(Wproj, f32)
    qr = np.asarray(q_ranges).astype(np.int64)
    kr = np.asarray(k_ranges).astype(np.int64)

    xT = np.ascontiguousarray(x.T).astype(np.float16)  # [H, S]

    # rope packs [D, S]: rows 0:HALF scale for x_lo terms, HALF:D for x_hi
    def pack(tab, wvec):
        return np.ascontiguousarray(
            np.concatenate([tab.T * wvec[:HALF, None], tab.T * wvec[HALF:, None]],
                           axis=0)).astype(f32)

    cospack_q, sinpack_q = pack(cos2, qw1), pack(sin2, qw1)
    cospack_k, sinpack_k = pack(cos2, kw1), pack(sin2, kw1)

    # ragged-range tile map in scores^T orientation: allowed[k, q]
    allowed = np.zeros((S, S), dtype=bool)
    covered = np.zeros((S,), dtype=bool)
    for r in range(qr.shape[0]):
        q0, q1 = int(qr[r, 0]), int(qr[r, 1])
        k0, k1 = int(kr[r, 0]), int(kr[r, 1])
        q0, q1 = max(q0, 0), min(q1, S)
        k0, k1 = max(k0, 0), min(k1, S)
        if q1 > q0:
            covered[q0:q1] = True
            if k1 > k0:
                allowed[k0:k1, q0:q1] = True

    n_kt = S // D
    n_sc = S // SC
    masks = []
    chunk_plan = []  # per sc: list of (kt, q0, q1, mask_id_or_None)
    uncov_needed = []
    for sc in range(n_sc):
        plan = []
        qs = slice(sc * SC, (sc + 1) * SC)
        for kt in range(n_kt):
            sub = allowed[kt * D:(kt + 1) * D, qs]
            if not sub.any():
                continue
            col_all = sub.all(axis=0)
            col_any = sub.any(axis=0)
            if np.array_equal(col_all, col_any):
                # columns are all-or-nothing: find contiguous window
                idx = np.nonzero(col_all)[0]
                q0, q1 = int(idx[0]), int(idx[-1]) + 1
                if col_all[q0:q1].all():
                    plan.append((kt, q0, q1, None))
                    continue
            masks.append(np.where(sub, np.float32(0), np.float32(NEG)))
            plan.append((kt, 0, SC, len(masks) - 1))
        chunk_plan.append(plan)
        has_keys = allowed[:, qs].any(axis=0)
        uncov_needed.append(None if has_keys.all()
                            else (~has_keys).astype(f32)[None, :])
        # PSUM accumulation requires the first tile to cover every column that
        # any later tile writes
        if plan:
            lo = min(p[1] for p in plan)
            hi = max(p[2] for p in plan)
            if not (plan[0][1] <= lo and plan[0][2] >= hi):
                # widen the first tile to full-with-mask
                kt0, a0, a1, m0 = plan[0]
                if m0 is None:
                    sub = allowed[kt0 * D:(kt0 + 1) * D, qs]
                    masks.append(np.where(sub, np.float32(0), np.float32(NEG)))
                    plan[0] = (kt0, 0, SC, len(masks) - 1)

    masks_arr = (np.ascontiguousarray(np.stack(masks)) if masks
                 else np.zeros((1, D, SC), f32))

    cov_arr = covered.astype(f32)[None, :]  # [1, S], for output zeroing

    per_core = []
    for c in range(NCORES):
        wkvq = np.ascontiguousarray(
            np.concatenate(
                [Wk[c * D:(c + 1) * D].T, Wv[c * D:(c + 1) * D].T,
                 Wq[c * NHQ * D:(c + 1) * NHQ * D].T], axis=1)).astype(np.float16)
        outc = H // NCORES
        wpt = np.ascontiguousarray(
            Wproj[c * outc:(c + 1) * outc].T).astype(np.float16)  # [HD, H//NCORES]
        per_core.append(dict(xT=xT, wkvq=wkvq, wpt=wpt,
                             cospack_q=cospack_q, sinpack_q=sinpack_q,
                             cospack_k=cospack_k, sinpack_k=sinpack_k,
                             masks=masks_arr))
    spec = dict(chunk_plan=chunk_plan, uncov=uncov_needed, covered=cov_arr,
                all_covered=bool(covered.all()))
    return per_core, spec


# ---------------------------------------------------------------------------
# Device program (identical on all cores; SPMD over inputs)
# ---------------------------------------------------------------------------
def _build_program(cfg, spec, n_masks, same_packs):
    import concourse.bass as bass
    import concourse.tile as tile
    from concourse import mybir

    f32 = mybir.dt.float32
    f32r = mybir.dt.float32r
    f16 = mybir.dt.float16
    AF = mybir.ActivationFunctionType

    S, H, HQ, HKV, D = cfg["S"], cfg["H"], cfg["HQ"], cfg["HKV"], cfg["D"]
    HALF = D // 2
    NHQ = HQ // NCORES
    HD = HQ * D
    n_ht = H // D
    n_kt = S // D
    n_sc = S // SC
    n_st = SC // D  # 128-token subtiles per chunk
    OUTC = H // NCORES  # output columns per core
    QKSCALE = float(1.0 / np.sqrt(D))
    chunk_plan = spec["chunk_plan"]
    uncov = spec["uncov"]

    # which k-chunk must be complete before attention(sc) can run
    maxk = []
    for sc in range(n_sc):
        plan = chunk_plan[sc]
        maxk.append(((max(kt for kt, _, _, _ in plan) * D) // SC) if plan else 0)

    # early chunk: smallest maxk (its attention+AllGather hoist into KV phase)
    early_sc = int(np.argmin(maxk))
    # remaining chunks: largest attention first so the tail chunk is small
    rest = sorted([sc for sc in range(n_sc) if sc != early_sc],
                  key=lambda sc: -len(chunk_plan[sc]))

    nc = bass.Bass(num_devices=NCORES)

    # register consts as AP so activation(bias=...) can resolve them
    _epst = nc.alloc_sbuf_tensor("const-float32-eps", [128, 1], f32)
    nc.gpsimd.memset(_epst.ap(), EPS)
    nc.const_aps.aps[(f32, EPS)] = _epst.ap()
    _negt = nc.alloc_sbuf_tensor("const-float32-neg1", [128, 1], f32)
    nc.gpsimd.memset(_negt.ap(), -1.0)
    nc.const_aps.aps[(f32, -1.0)] = _negt.ap()
    nc.all_engine_barrier()

    xT_d = nc.dram_tensor("xT", [H, S], f16, kind="ExternalInput")
    wkvq_d = nc.dram_tensor("wkvq", [H, (2 + NHQ) * D], f16, kind="ExternalInput")
    wpt_d = nc.dram_tensor("wpt", [HD, OUTC], f16, kind="ExternalInput")
    cq_d = nc.dram_tensor("cospack_q", [D, S], f32, kind="ExternalInput")
    sq_d = nc.dram_tensor("sinpack_q", [D, S], f32, kind="ExternalInput")
    ck_d = nc.dram_tensor("cospack_k", [D, S], f32, kind="ExternalInput")
    sk_d = nc.dram_tensor("sinpack_k", [D, S], f32, kind="ExternalInput")
    masks_d = nc.dram_tensor("masks", [n_masks, D, SC], f32, kind="ExternalInput")
    out_d = nc.dram_tensor("out", [OUTC, S], f32, kind="ExternalOutput")

    r_dram = nc.dram_tensor("r_scratch", [1, S], f32)
    sync_in = nc.dram_tensor("sync_in", [1, 128], f32)
    sync_out = nc.dram_tensor("sync_out", [NCORES, 128], f32, addr_space="Shared")
    ag_in = [nc.dram_tensor(f"ag_in_{j}", [NHQ * D, SC], f16) for j in range(n_sc)]
    ag_out = [nc.dram_tensor(f"ag_out_{j}", [HD, SC], f16, addr_space="Shared")
              for j in range(n_sc)]

    uncov_d = None
    if any(u is not None for u in uncov):
        uncov_d = nc.dram_tensor("uncov", [1, S], f32, kind="ExternalInput")

    ident_d = nc.inline_tensor(np.eye(D, dtype=np.float32), name="ident128")
    ones_d = nc.inline_tensor(np.ones((D, 1), dtype=np.float32), name="ones128")
    onesr_d = nc.inline_tensor(np.ones((1, D), dtype=np.float32), name="ones1x128")

    from contextlib import ExitStack
    with tile.TileContext(nc) as tc, ExitStack() as ctx:
        pool = lambda *a, **k: ctx.enter_context(tc.tile_pool(*a, **k))
        const_p = pool(name="const", bufs=1)
        w_p = pool(name="wkvq", bufs=n_ht)
        wpt_p = pool(name="wpt", bufs=HD // D)
        big_p = pool(name="big", bufs=1)
        x_p = pool(name="x", bufs=4)        # paired KV loads [D, 2*SC]
        xq_p = pool(name="xq", bufs=n_ht)   # QA phase: whole chunk resident
        sq_p = pool(name="sq", bufs=3)
        trig_p = pool(name="trig", bufs=2)
        rope_p = pool(name="rope", bufs=4)
        tmp_p = pool(name="tmp", bufs=2)
        qh_p = pool(name="qh", bufs=4)
        pexp_p = pool(name="pexp", bufs=6)
        row_p = pool(name="row", bufs=4)
        rb_p = pool(name="rb", bufs=2)
        at_p = pool(name="at", bufs=2)
        lt_p = pool(name="lt", bufs=6)
        os_p = pool(name="os", bufs=2)
        any_masks = any(mid is not None for plan in chunk_plan
                        for _, _, _, mid in plan)
        mask_p = pool(name="mask", bufs=1) if any_masks else None
        psN = pool(name="psN", bufs=8, space="PSUM")

        ident = const_p.tile([D, D], f32r)
        nc.sync.dma_start(ident[:], ident_d.ap().bitcast(f32r))
        ones = const_p.tile([D, 1], f32r)
        nc.sync.dma_start(ones[:], ones_d.ap().bitcast(f32r))
        onesr = const_p.tile([1, D], f32r)
        nc.sync.dma_start(onesr[:], onesr_d.ap().bitcast(f32r))
        ones16 = const_p.tile([D, 1], f16)
        nc.vector.tensor_copy(ones16[:], ones[:].bitcast(f32))

        wkvq_sb = []
        for t in range(n_ht):
            w = w_p.tile([D, (2 + NHQ) * D], f16, tag="w")
            nc.sync.dma_start(w[:], wkvq_d[t * D:(t + 1) * D, :])
            wkvq_sb.append(w)
        wpt_sb = []

        def load_wpt():
            for t in range(HD // D):
                w = wpt_p.tile([D, OUTC], f16, tag="wp", name=f"wpt{t}")
                nc.sync.dma_start(w[:], wpt_d[t * D:(t + 1) * D, :])
                wpt_sb.append(w)

        # tiny collective up-front: absorbs inter-core dispatch skew while
        # weights stream in, so the first real AllGather isn't a barrier
        nc.gpsimd.collective_compute(
            "AllGather", mybir.AluOpType.bypass,
            replica_groups=[list(range(NCORES))],
            ins=[sync_in.ap()], outs=[sync_out.ap()],
        )

        khatT = big_p.tile([D, S], f16, tag="khat")   # [d, token]
        v_sb = big_p.tile([D, S], f16, tag="v")       # [token(kt-major), d]
        r_pp = big_p.tile([D, n_kt], f32, tag="rpp")  # per-partition r

        uncov_sb = None
        if uncov_d is not None:
            uncov_sb = big_p.tile([1, S], f32, tag="uncov")
            nc.sync.dma_start(uncov_sb[:], uncov_d[:, :])

        def rope_block(psrc, cos_t, sin_t, dst_ap, scale_sb):
            # cospack rows [cos*w_lo ; cos*w_hi], sinpack [sin*w_lo ; sin*w_hi]
            t1 = rope_p.tile([HALF, SC], f32, tag="rp")
            t2 = rope_p.tile([HALF, SC], f32, tag="rp")
            t3 = rope_p.tile([HALF, SC], f32, tag="rp")
            t4 = rope_p.tile([HALF, SC], f32, tag="rp")
            nc.vector.tensor_mul(t1[:], psrc[0:HALF, :], cos_t[0:HALF, :])
            nc.vector.tensor_mul(t2[:], psrc[HALF:D, :], sin_t[HALF:D, :])
            nc.vector.tensor_mul(t3[:], psrc[HALF:D, :], cos_t[HALF:D, :])
            nc.vector.tensor_mul(t4[:], psrc[0:HALF, :], sin_t[0:HALF, :])
            tmp = tmp_p.tile([D, SC], f32, tag="ropetmp")
            nc.vector.tensor_sub(tmp[0:HALF, :], t1[:], t2[:])
            nc.vector.tensor_add(tmp[HALF:D, :], t3[:], t4[:])
            nc.vector.tensor_mul(dst_ap, tmp[:], scale_sb)

        def rms_scale(p_raw):
            # 1/rms per token, broadcast to [D, SC] via outer-product matmul
            sq = sq_p.tile([D, SC], f16, tag="sq")
            nc.scalar.activation(sq[:], p_raw[:], AF.Square)
            pss = psN.tile([1, SC], f32, tag="b")
            nc.tensor.matmul(pss[:], ones16[:], sq[:], start=True, stop=True)
            tvar = row_p.tile([1, SC], f32, tag="row")
            nc.scalar.activation(tvar[:], pss[:], AF.Ln, scale=1.0 / D, bias=EPS)
            rq = row_p.tile([1, SC], f32r, tag="rowr")
            nc.scalar.activation(rq[:], tvar[:], AF.Exp, scale=-0.5)
            prb = psN.tile([D, SC], f32, tag="b")
            nc.tensor.matmul(prb[:], onesr[:], rq[:], start=True, stop=True)
            rb = rb_p.tile([D, SC], f32, tag="rb")
            nc.scalar.copy(rb[:], prb[:])
            return rb

        n_ct = OUTC // D

        def emit_proj(psc):
            pssl = slice(psc * SC, (psc + 1) * SC)
            po = [psN.tile([D, SC], f32, tag="b", name=f"po{_j}")
                  for _j in range(n_ct)]
            for t2 in range(HD // (2 * D)):
                lt = lt_p.tile([D, 2 * SC], f16, tag="lt")
                nc.gpsimd.dma_start(
                    lt[:],
                    ag_out[psc][2 * t2 * D:(2 * t2 + 2) * D, :]
                    .rearrange("(j p) n -> p j n", j=2))
                for jj in range(2):
                    t = 2 * t2 + jj
                    ltv = lt[:, jj * SC:(jj + 1) * SC]
                    for j in range(n_ct):
                        nc.tensor.matmul(po[j][:], wpt_sb[t][:, j * D:(j + 1) * D],
                                         ltv, start=(t == 0),
                                         stop=(t == HD // D - 1))
            for j in range(n_ct):
                ob = os_p.tile([D, SC], f32, tag="os")
                nc.vector.tensor_copy(ob[:], po[j][:])
                nc.sync.dma_start(out_d[j * D:(j + 1) * D, pssl], ob[:])

        # ------------------------- KV phase ------------------------------
        def emit_kv(sc):
            ssl = slice(sc * SC, (sc + 1) * SC)
            pk = psN.tile([D, SC], f32, tag="b")
            pv = psN.tile([D, SC], f32, tag="b")
            pss = psN.tile([1, SC], f32, tag="b")
            for h2 in range(n_ht // 2):
                xt = x_p.tile([D, 2 * SC], f16, tag="x")
                nc.sync.dma_start(
                    xt[:],
                    xT_d[2 * h2 * D:(2 * h2 + 2) * D, ssl]
                    .rearrange("(j p) n -> p j n", j=2))
                for jj in range(2):
                    ht = 2 * h2 + jj
                    xv = xt[:, jj * SC:(jj + 1) * SC]
                    st, sp = ht == 0, ht == n_ht - 1
                    nc.tensor.matmul(pk[:], wkvq_sb[ht][:, 0:D], xv,
                                     start=st, stop=sp)
                    nc.tensor.matmul(pv[:], wkvq_sb[ht][:, D:2 * D], xv,
                                     start=st, stop=sp)
                    sqx = sq_p.tile([D, SC], f16, tag="sqx")
                    nc.vector.tensor_mul(sqx[:], xv, xv)
                    nc.tensor.matmul(pss[:], ones16[:], sqx[:], start=st, stop=sp)
            # r chunk: 1/rms(x) per token
            tvar = row_p.tile([1, SC], f32, tag="row")
            nc.scalar.activation(tvar[:], pss[:], AF.Ln, scale=1.0 / H, bias=EPS)
            r_chunk = row_p.tile([1, SC], f32, tag="row")
            nc.scalar.activation(r_chunk[:], tvar[:], AF.Exp, scale=-0.5)
            # repack [1, SC] -> [D, n_st] per-partition layout via DRAM
            nc.sync.dma_start(r_dram[0:1, ssl], r_chunk[:])
            nc.sync.dma_start(
                r_pp[:, sc * n_st:(sc + 1) * n_st],
                r_dram[0:1, ssl].rearrange("o (j p) -> (o p) j", p=D))
            # k: rms + rope
            ckt = trig_p.tile([D, SC], f32, tag="ck")
            skt = trig_p.tile([D, SC], f32, tag="sk")
            nc.gpsimd.dma_start(ckt[:], ck_d[:, ssl])
            nc.gpsimd.dma_start(skt[:], sk_d[:, ssl])
            rb = rms_scale(pk)
            rope_block(pk, ckt, skt, khatT[:, ssl], rb[:])
            # v: copy then transpose to [token, d], scaled by r
            vt = tmp_p.tile([D, SC], f32r, tag="vt")
            nc.vector.tensor_copy(vt[:], pv[:])
            for j in range(n_st):
                ptr = psN.tile([D, D], f32r, tag="b")
                nc.tensor.transpose(ptr[:], vt[:, j * D:(j + 1) * D], ident[:])
                kt = sc * n_st + j
                nc.scalar.activation(v_sb[:, kt * D:(kt + 1) * D],
                                     ptr[:].bitcast(f32), AF.Copy,
                                     scale=r_pp[:, kt:kt + 1])

        # ------------------------- QA phase ------------------------------
        proj_queue = []

        def drain_proj(keep):
            while len(proj_queue) > keep:
                emit_proj(proj_queue.pop(0))

        def emit_qa(sc):
            ssl = slice(sc * SC, (sc + 1) * SC)
            plan = chunk_plan[sc]
            # load whole x chunk resident (paired rows)
            xts = []
            for h2 in range(n_ht // 2):
                xt = xq_p.tile([D, 2 * SC], f16, tag="xq")
                nc.sync.dma_start(
                    xt[:],
                    xT_d[2 * h2 * D:(2 * h2 + 2) * D, ssl]
                    .rearrange("(j p) n -> p j n", j=2))
                xts.append(xt)
            if same_packs:
                cqt = trig_p.tile([D, SC], f32, tag="ck")
                sqt = trig_p.tile([D, SC], f32, tag="sk")
                nc.gpsimd.dma_start(cqt[:], ck_d[:, ssl])
                nc.gpsimd.dma_start(sqt[:], sk_d[:, ssl])
            else:
                cqt = trig_p.tile([D, SC], f32, tag="ck")
                sqt = trig_p.tile([D, SC], f32, tag="sk")
                nc.gpsimd.dma_start(cqt[:], cq_d[:, ssl])
                nc.gpsimd.dma_start(sqt[:], sq_d[:, ssl])

            pden3 = psN.tile([D, SC], f32, tag="b")  # head h uses row 32*h

            def emit_qproj(h):
                pq = psN.tile([D, SC], f32, tag="b", name=f"pq{h}")
                for ht in range(n_ht):
                    st, sp = ht == 0, ht == n_ht - 1
                    nc.tensor.matmul(pq[:], wkvq_sb[ht][:, (2 + h) * D:(3 + h) * D],
                                     xts[ht // 2][:, (ht % 2) * SC:(ht % 2 + 1) * SC],
                                     start=st, stop=sp)
                return pq

            def emit_qrope(pq):
                rbq = rms_scale(pq)
                qh = qh_p.tile([D, SC], f16, tag="qh")
                rope_block(pq, cqt, sqt, qh[:], rbq[:])
                return qh

            def emit_attention_head(h, qh):
                pattn = psN.tile([D, SC], f32, tag="b")
                pes = {}

                def emit_qk(i):
                    kt, a0, a1, mid = plan[i]
                    ps = psN.tile([D, SC], f32, tag="b")
                    nc.tensor.matmul(ps[:, a0:a1], khatT[:, kt * D:(kt + 1) * D],
                                     qh[:, a0:a1], start=True, stop=True)
                    if mid is not None:
                        mt = mask_p.tile([D, SC], f32, tag="m")
                        nc.sync.dma_start(mt[:], masks_d[mid, :, :])
                        nc.vector.tensor_add(ps[:, a0:a1], ps[:, a0:a1],
                                             mt[:, a0:a1])
                    pe = pexp_p.tile([D, SC], f16, tag="pe")
                    nc.scalar.activation(pe[:, a0:a1], ps[:, a0:a1], AF.Exp,
                                         scale=QKSCALE, bias=-1.0)
                    pes[i] = pe

                def emit_pv(i):
                    kt, a0, a1, mid = plan[i]
                    pe = pes.pop(i)
                    first, last = i == 0, i == len(plan) - 1
                    nc.tensor.matmul(pattn[:, a0:a1], v_sb[:, kt * D:(kt + 1) * D],
                                     pe[:, a0:a1], start=first, stop=last)
                    nc.tensor.matmul(pden3[32 * h:32 * h + 1, a0:a1], ones16[:], pe[:, a0:a1],
                                     start=first, stop=last)

                LAG = 3
                for i in range(len(plan)):
                    emit_qk(i)
                    if i >= LAG:
                        emit_pv(i - LAG)
                for i in range(max(0, len(plan) - LAG), len(plan)):
                    emit_pv(i)

                at = at_p.tile([D, SC], f16, tag="at")
                if not plan:
                    nc.vector.memset(at[:], 0.0)
                else:
                    if uncov[sc] is not None:
                        nc.vector.tensor_add(pden3[32 * h:32 * h + 1, :], pden3[32 * h:32 * h + 1, :],
                                             uncov_sb[0:1, ssl])
                    dln = row_p.tile([1, SC], f32, tag="row")
                    nc.scalar.activation(dln[:], pden3[32 * h:32 * h + 1, :], AF.Ln)
                    rec = row_p.tile([1, SC], f32r, tag="rowr")
                    nc.scalar.activation(rec[:], dln[:], AF.Exp, scale=-1.0)
                    prb = psN.tile([D, SC], f32, tag="b")
                    nc.tensor.matmul(prb[:], onesr[:], rec[:], start=True, stop=True)
                    rb2 = rb_p.tile([D, SC], f32, tag="rb")
                    nc.vector.tensor_copy(rb2[:], prb[:])
                    nc.vector.tensor_mul(at[:], pattn[:], rb2[:])
                nc.sync.dma_start(ag_in[sc][h * D:(h + 1) * D, :], at[:])

            # pipeline: qproj(0) -> [qproj(h+1) overlaps rope(h)+attn(h)]
            pqs = {0: emit_qproj(0)}
            for h in range(NHQ):
                qh = emit_qrope(pqs.pop(h))
                if h + 1 < NHQ:
                    pqs[h + 1] = emit_qproj(h + 1)
                emit_attention_head(h, qh)
            nc.gpsimd.collective_compute(
                "AllGather", mybir.AluOpType.bypass,
                replica_groups=[list(range(NCORES))],
                ins=[ag_in[sc].ap()], outs=[ag_out[sc].ap()],
            )
            proj_queue.append(sc)

        # ---------------- main schedule ----------------
        load_wpt()
        kv_done = -1
        for sc in range(n_sc):
            emit_kv(sc)
            kv_done = sc
            if kv_done == maxk[early_sc] and sc < n_sc - 1:
                emit_qa(early_sc)
        for sc in rest:
            drain_proj(1)
            emit_qa(sc)
        drain_proj(0)

    return nc


def build_and_run(x, cos, sin, pre_norm_w, q_norm_w, k_norm_w, Wq, Wk, Wv,
                  Wproj, q_ranges, k_ranges, cfg=None, trace=False,
                  trace_kwargs=None):
    from concourse.bass_utils import run_bass_kernel_spmd

    cfg = cfg or FULL_CFG
    per_core, spec = _host_prep(x, cos, sin, pre_norm_w, q_norm_w, k_norm_w,
                                Wq, Wk, Wv, Wproj, q_ranges, k_ranges, cfg)
    n_masks = per_core[0]["masks"].shape[0]
    same_packs = (np.array_equal(per_core[0]["cospack_q"], per_core[0]["cospack_k"])
                  and np.array_equal(per_core[0]["sinpack_q"], per_core[0]["sinpack_k"]))
    nc = _build_program(cfg, spec, n_masks, same_packs)
    _patch_bass(nc)

    in_maps = []
    for c in range(NCORES):
        m = dict(per_core[c])
        if any(u is not None for u in spec["uncov"]):
            S = cfg["S"]
            ua = np.zeros((1, S), np.float32)
            for sc, u in enumerate(spec["uncov"]):
                if u is not None:
                    ua[0, sc * SC:(sc + 1) * SC] = u
            m["uncov"] = ua
        in_maps.append(m)

    kw = {}
    if trace:
        kw = dict(trace=True, trace_kwargs=trace_kwargs or {})
    res = run_bass_kernel_spmd(nc, in_maps, core_ids=list(range(NCORES)), **kw)
    out = np.concatenate([res.results[c]["out"] for c in range(NCORES)], axis=0).T
    if not spec["all_covered"]:
        out = out * spec["covered"].T  # zero uncovered rows
    return out, res


def kernel(**inputs):
    out, _ = build_and_run(**inputs)
    return out


# revision 17
# speedup vs baseline: 1.1327x; 1.0241x over previous
"""Trainium2 Bass kernel for nn_Attention_41102837023186 (sparse GQA attention).

Head-tensor-parallel over 8 NeuronCores: core c owns q heads [3c, 3c+3) and
kv head c. Structure: KV projections+rope for all token chunks first, then per
token chunk (largest-attention first) Q projection+rope+block-sparse attention
with column-windowed partial tiles, chunked AllGather of attention outputs,
and the Wproj column block overlapped with later chunks.

kernel(**inputs) takes the FULL unsharded inputs and returns the FULL output.
"""

import numpy as np

FULL_CFG = dict(S=3072, H=3072, HQ=24, HKV=8, D=128)
NCORES = 8
SC = 512  # token chunk (free-dim tile)
EPS = 1e-6
NEG = -1e30

_uid = [0]


# ---------------------------------------------------------------------------
# BIR post-fix: this walrus build accepts only ONE sem wait per instruction;
# Tile emits more (tail drain, DMA fan-ins). Split overflow waits onto
# preceding NoOp instructions on the same engine.
# ---------------------------------------------------------------------------
def _fix_bir_json_bytes(raw: bytes) -> bytes:
    import json as _json

    m = _json.loads(raw)
    changed = False
    for f in m.get("functions", []):
        for blk in f.get("blocks", []):
            out = []
            for inst in blk["instructions"]:
                si = inst.get("sync_info") or {}
                waits = si.get("on_wait") or []
                if len(waits) > 1:
                    changed = True
                    for w in waits[:-1]:
                        _uid[0] += 1
                        out.append(
                            {
                                "name": f"I-waitsplit-{_uid[0]}",
                                "opcode": "NoOp",
                                "engine": inst["engine"],
                                "ins": [],
                                "outs": [],
                                "debug": inst.get("debug", 0),
                                "sync_info": {"on_update": [], "on_wait": [w]},
                            }
                        )
                    si = dict(si)
                    si["on_wait"] = waits[-1:]
                    inst = dict(inst)
                    inst["sync_info"] = si
                out.append(inst)
            blk["instructions"] = out
    if not changed:
        return raw
    return _json.dumps(m).encode()


def _patch_bass(nc):
    import types

    orig = nc.to_json_bytes

    def patched(self):
        return _fix_bir_json_bytes(orig())

    nc.to_json_bytes = types.MethodType(patched, nc)
    return nc


# ---------------------------------------------------------------------------
# Host-side prep: fold norm weights, transpose layouts, range -> tile plan
# ---------------------------------------------------------------------------
def _host_prep(x, cos, sin, pre_norm_w, q_norm_w, k_norm_w, Wq, Wk, Wv, Wproj,
               q_ranges, k_ranges, cfg):
    S, H, HQ, HKV, D = cfg["S"], cfg["H"], cfg["HQ"], cfg["HKV"], cfg["D"]
    HALF = D // 2
    NHQ = HQ // NCORES
    f32 = np.float32

    x = np.asarray(x, f32)
    cos2 = np.asarray(cos, f32).reshape(S, HALF)
    sin2 = np.asarray(sin, f32).reshape(S, HALF)
    w1 = (np.asarray(pre_norm_w, f32) + 1.0)
    qw1 = (np.asarray(q_norm_w, f32) + 1.0)
    kw1 = (np.asarray(k_norm_w, f32) + 1.0)
    Wq = np.asarray(Wq, f32) * w1[None, :]
    Wk = np.asarray(Wk, f32) * w1[None, :]
    Wv = np.asarray(Wv, f32) * w1[None, :]
    Wproj = np.asarray(Wproj, f32)
    qr = np.asarray(q_ranges).astype(np.int64)
    kr = np.asarray(k_ranges).astype(np.int64)

    xT = np.ascontiguousarray(x.T).astype(np.float16)  # [H, S]

    # rope packs [D, S]: rows 0:HALF scale for x_lo terms, HALF:D for x_hi
    def pack(tab, wvec):
        return np.ascontiguousarray(
            np.concatenate([tab.T * wvec[:HALF, None], tab.T * wvec[HALF:, None]],
                           axis=0)).astype(f32)

    cospack_q, sinpack_q = pack(cos2, qw1), pack(sin2, qw1)
    cospack_k, sinpack_k = pack(cos2, kw1), pack(sin2, kw1)

    # ragged-range tile map in scores^T orientation: allowed[k, q]
    allowed = np.zeros((S, S), dtype=bool)
    covered = np.zeros((S,), dtype=bool)
    for r in range(qr.shape[0]):
        q0, q1 = int(qr[r, 0]), int(qr[r, 1])
        k0, k1 = int(kr[r, 0]), int(kr[r, 1])
        q0, q1 = max(q0, 0), min(q1, S)
        k0, k1 = max(k0, 0), min(k1, S)
        if q1 > q0:
            covered[q0:q1] = True
            if k1 > k0:
                allowed[k0:k1, q0:q1] = True

    n_kt = S // D
    n_sc = S // SC
    masks = []
    chunk_plan = []  # per sc: list of (kt, q0, q1, mask_id_or_None)
    uncov_needed = []
    for sc in range(n_sc):
        plan = []
        qs = slice(sc * SC, (sc + 1) * SC)
        for kt in range(n_kt):
            sub = allowed[kt * D:(kt + 1) * D, qs]
            if not sub.any():
                continue
            col_all = sub.all(axis=0)
            col_any = sub.any(axis=0)
            if np.array_equal(col_all, col_any):
                # columns are all-or-nothing: find contiguous window
                idx = np.nonzero(col_all)[0]
                q0, q1 = int(idx[0]), int(idx[-1]) + 1
                if col_all[q0:q1].all():
                    plan.append((kt, q0, q1, None))
                    continue
            masks.append(np.where(sub, np.float32(0), np.float32(NEG)))
            plan.append((kt, 0, SC, len(masks) - 1))
        chunk_plan.append(plan)
        has_keys = allowed[:, qs].any(axis=0)
        uncov_needed.append(None if has_keys.all()
                            else (~has_keys).astype(f32)[None, :])
        # PSUM accumulation requires the first tile to cover every column that
        # any later tile writes
        if plan:
            lo = min(p[1] for p in plan)
            hi = max(p[2] for p in plan)
            if not (plan[0][1] <= lo and plan[0][2] >= hi):
                # widen the first tile to full-with-mask
                kt0, a0, a1, m0 = plan[0]
                if m0 is None:
                    sub = allowed[kt0 * D:(kt0 + 1) * D, qs]
                    masks.append(np.where(sub, np.float32(0), np.float32(NEG)))
                    plan[0] = (kt0, 0, SC, len(masks) - 1)

    masks_arr = (np.ascontiguousarray(np.stack(masks)) if masks
                 else np.zeros((1, D, SC), f32))

    cov_arr = covered.astype(f32)[None, :]  # [1, S], for output zeroing

    per_core = []
    for c in range(NCORES):
        wkvq = np.ascontiguousarray(
            np.concatenate(
                [Wk[c * D:(c + 1) * D].T, Wv[c * D:(c + 1) * D].T,
                 Wq[c * NHQ * D:(c + 1) * NHQ * D].T], axis=1)).astype(np.float16)
        outc = H // NCORES
        wpt = np.ascontiguousarray(
            Wproj[c * outc:(c + 1) * outc].T).astype(np.float16)  # [HD, H//NCORES]
        per_core.append(dict(xT=xT, wkvq=wkvq, wpt=wpt,
                             cospack_q=cospack_q, sinpack_q=sinpack_q,
                             cospack_k=cospack_k, sinpack_k=sinpack_k,
                             masks=masks_arr))
    spec = dict(chunk_plan=chunk_plan, uncov=uncov_needed, covered=cov_arr,
                all_covered=bool(covered.all()))
    return per_core, spec


# ---------------------------------------------------------------------------
# Device program (identical on all cores; SPMD over inputs)
# ---------------------------------------------------------------------------
def _build_program(cfg, spec, n_masks, same_packs):
    import concourse.bass as bass
    import concourse.tile as tile
    from concourse import mybir

    f32 = mybir.dt.float32
    f32r = mybir.dt.float32r
    f16 = mybir.dt.float16
    AF = mybir.ActivationFunctionType

    S, H, HQ, HKV, D = cfg["S"], cfg["H"], cfg["HQ"], cfg["HKV"], cfg["D"]
    HALF = D // 2
    NHQ = HQ // NCORES
    HD = HQ * D
    n_ht = H // D
    n_kt = S // D
    n_sc = S // SC
    n_st = SC // D  # 128-token subtiles per chunk
    OUTC = H // NCORES  # output columns per core
    QKSCALE = float(1.0 / np.sqrt(D))
    chunk_plan = spec["chunk_plan"]
    uncov = spec["uncov"]

    # which k-chunk must be complete before attention(sc) can run
    maxk = []
    for sc in range(n_sc):
        plan = chunk_plan[sc]
        maxk.append(((max(kt for kt, _, _, _ in plan) * D) // SC) if plan else 0)

    # early chunk: smallest maxk (its attention+AllGather hoist into KV phase)
    early_sc = int(np.argmin(maxk))
    # remaining chunks: largest attention first so the tail chunk is small
    rest = sorted([sc for sc in range(n_sc) if sc != early_sc],
                  key=lambda sc: -len(chunk_plan[sc]))

    nc = bass.Bass(num_devices=NCORES)

    # register consts as AP so activation(bias=...) can resolve them
    _epst = nc.alloc_sbuf_tensor("const-float32-eps", [128, 1], f32)
    nc.gpsimd.memset(_epst.ap(), EPS)
    nc.const_aps.aps[(f32, EPS)] = _epst.ap()
    _negt = nc.alloc_sbuf_tensor("const-float32-neg1", [128, 1], f32)
    nc.gpsimd.memset(_negt.ap(), -1.0)
    nc.const_aps.aps[(f32, -1.0)] = _negt.ap()
    nc.all_engine_barrier()

    xT_d = nc.dram_tensor("xT", [H, S], f16, kind="ExternalInput")
    wkvq_d = nc.dram_tensor("wkvq", [H, (2 + NHQ) * D], f16, kind="ExternalInput")
    wpt_d = nc.dram_tensor("wpt", [HD, OUTC], f16, kind="ExternalInput")
    cq_d = nc.dram_tensor("cospack_q", [D, S], f32, kind="ExternalInput")
    sq_d = nc.dram_tensor("sinpack_q", [D, S], f32, kind="ExternalInput")
    ck_d = nc.dram_tensor("cospack_k", [D, S], f32, kind="ExternalInput")
    sk_d = nc.dram_tensor("sinpack_k", [D, S], f32, kind="ExternalInput")
    masks_d = nc.dram_tensor("masks", [n_masks, D, SC], f32, kind="ExternalInput")
    out_d = nc.dram_tensor("out", [OUTC, S], f32, kind="ExternalOutput")

    r_dram = nc.dram_tensor("r_scratch", [1, S], f32)
    sync_in = nc.dram_tensor("sync_in", [1, 128], f32)
    sync_out = nc.dram_tensor("sync_out", [NCORES, 128], f32, addr_space="Shared")
    ag_in = [nc.dram_tensor(f"ag_in_{j}", [NHQ * D, SC], f16) for j in range(n_sc)]
    ag_out = [nc.dram_tensor(f"ag_out_{j}", [HD, SC], f16, addr_space="Shared")
              for j in range(n_sc)]

    uncov_d = None
    if any(u is not None for u in uncov):
        uncov_d = nc.dram_tensor("uncov", [1, S], f32, kind="ExternalInput")

    ident_d = nc.inline_tensor(np.eye(D, dtype=np.float32), name="ident128")
    ones_d = nc.inline_tensor(np.ones((D, 1), dtype=np.float32), name="ones128")
    onesr_d = nc.inline_tensor(np.ones((1, D), dtype=np.float32), name="ones1x128")

    from contextlib import ExitStack
    with tile.TileContext(nc) as tc, ExitStack() as ctx:
        pool = lambda *a, **k: ctx.enter_context(tc.tile_pool(*a, **k))
        const_p = pool(name="const", bufs=1)
        w_p = pool(name="wkvq", bufs=n_ht)
        wpt_p = pool(name="wpt", bufs=HD // D)
        big_p = pool(name="big", bufs=1)
        x_p = pool(name="x", bufs=4)        # paired KV loads [D, 2*SC]
        xq_p = pool(name="xq", bufs=n_ht)   # QA phase: whole chunk resident
        sq_p = pool(name="sq", bufs=3)
        trig_p = pool(name="trig", bufs=2)
        rope_p = pool(name="rope", bufs=4)
        tmp_p = pool(name="tmp", bufs=2)
        qh_p = pool(name="qh", bufs=4)
        pexp_p = pool(name="pexp", bufs=6)
        row_p = pool(name="row", bufs=4)
        rb_p = pool(name="rb", bufs=2)
        at_p = pool(name="at", bufs=2)
        lt_p = pool(name="lt", bufs=6)
        os_p = pool(name="os", bufs=2)
        any_masks = any(mid is not None for plan in chunk_plan
                        for _, _, _, mid in plan)
        mask_p = pool(name="mask", bufs=1) if any_masks else None
        psN = pool(name="psN", bufs=8, space="PSUM")

        ident = const_p.tile([D, D], f32r)
        nc.sync.dma_start(ident[:], ident_d.ap().bitcast(f32r))
        ones = const_p.tile([D, 1], f32r)
        nc.sync.dma_start(ones[:], ones_d.ap().bitcast(f32r))
        onesr = const_p.tile([1, D], f32r)
        nc.sync.dma_start(onesr[:], onesr_d.ap().bitcast(f32r))
        ones16 = const_p.tile([D, 1], f16)
        nc.vector.tensor_copy(ones16[:], ones[:].bitcast(f32))

        wkvq_sb = []
        for t in range(n_ht):
            w = w_p.tile([D, (2 + NHQ) * D], f16, tag="w")
            nc.sync.dma_start(w[:], wkvq_d[t * D:(t + 1) * D, :])
            wkvq_sb.append(w)
        wpt_sb = []

        def load_wpt():
            for t in range(HD // D):
                w = wpt_p.tile([D, OUTC], f16, tag="wp", name=f"wpt{t}")
                nc.sync.dma_start(w[:], wpt_d[t * D:(t + 1) * D, :])
                wpt_sb.append(w)

        # tiny collective up-front: absorbs inter-core dispatch skew while
        # weights stream in, so the first real AllGather isn't a barrier
        nc.gpsimd.collective_compute(
            "AllGather", mybir.AluOpType.bypass,
            replica_groups=[list(range(NCORES))],
            ins=[sync_in.ap()], outs=[sync_out.ap()],
        )

        khatT = big_p.tile([D, S], f16, tag="khat")   # [d, token]
        v_sb = big_p.tile([D, S], f16, tag="v")       # [token(kt-major), d]
        r_pp = big_p.tile([D, n_kt], f32, tag="rpp")  # per-partition r

        uncov_sb = None
        if uncov_d is not None:
            uncov_sb = big_p.tile([1, S], f32, tag="uncov")
            nc.sync.dma_start(uncov_sb[:], uncov_d[:, :])

        def rope_block(psrc, cos_t, sin_t, dst_ap, scale_sb):
            # cospack rows [cos*w_lo ; cos*w_hi], sinpack [sin*w_lo ; sin*w_hi]
            t1 = rope_p.tile([HALF, SC], f32, tag="rp")
            t2 = rope_p.tile([HALF, SC], f32, tag="rp")
            t3 = rope_p.tile([HALF, SC], f32, tag="rp")
            t4 = rope_p.tile([HALF, SC], f32, tag="rp")
            nc.vector.tensor_mul(t1[:], psrc[0:HALF, :], cos_t[0:HALF, :])
            nc.vector.tensor_mul(t2[:], psrc[HALF:D, :], sin_t[HALF:D, :])
            nc.vector.tensor_mul(t3[:], psrc[HALF:D, :], cos_t[HALF:D, :])
            nc.vector.tensor_mul(t4[:], psrc[0:HALF, :], sin_t[0:HALF, :])
            tmp = tmp_p.tile([D, SC], f32, tag="ropetmp")
            nc.vector.tensor_sub(tmp[0:HALF, :], t1[:], t2[:])
            nc.vector.tensor_add(tmp[HALF:D, :], t3[:], t4[:])
            nc.vector.tensor_mul(dst_ap, tmp[:], scale_sb)

        def rms_scale(p_raw):
            # 1/rms per token, broadcast to [D, SC] via outer-product matmul
            sq = sq_p.tile([D, SC], f16, tag="sq")
            nc.scalar.activation(sq[:], p_raw[:], AF.Square)
            pss = psN.tile([1, SC], f32, tag="b")
            nc.tensor.matmul(pss[:], ones16[:], sq[:], start=True, stop=True)
            tvar = row_p.tile([1, SC], f32, tag="row")
            nc.scalar.activation(tvar[:], pss[:], AF.Ln, scale=1.0 / D, bias=EPS)
            rq = row_p.tile([1, SC], f32r, tag="rowr")
            nc.scalar.activation(rq[:], tvar[:], AF.Exp, scale=-0.5)
            prb = psN.tile([D, SC], f32, tag="b")
            nc.tensor.matmul(prb[:], onesr[:], rq[:], start=True, stop=True)
            rb = rb_p.tile([D, SC], f32, tag="rb")
            nc.scalar.copy(rb[:], prb[:])
            return rb

        n_ct = OUTC // D

        def emit_proj(psc):
            pssl = slice(psc * SC, (psc + 1) * SC)
            po = [psN.tile([D, SC], f32, tag="b", name=f"po{_j}")
                  for _j in range(n_ct)]
            for t2 in range(HD // (2 * D)):
                lt = lt_p.tile([D, 2 * SC], f16, tag="lt")
                nc.gpsimd.dma_start(
                    lt[:],
                    ag_out[psc][2 * t2 * D:(2 * t2 + 2) * D, :]
                    .rearrange("(j p) n -> p j n", j=2))
                for jj in range(2):
                    t = 2 * t2 + jj
                    ltv = lt[:, jj * SC:(jj + 1) * SC]
                    for j in range(n_ct):
                        nc.tensor.matmul(po[j][:], wpt_sb[t][:, j * D:(j + 1) * D],
                                         ltv, start=(t == 0),
                                         stop=(t == HD // D - 1))
            for j in range(n_ct):
                ob = os_p.tile([D, SC], f32, tag="os")
                nc.vector.tensor_copy(ob[:], po[j][:])
                nc.sync.dma_start(out_d[j * D:(j + 1) * D, pssl], ob[:])

        # ------------------------- KV phase ------------------------------
        # split into the matmul burst and the tail (rms/rope/v) so the tail's
        # PE ops (broadcast matmul, transposes) are emitted a chunk later and
        # never stall the PE on the ACT row chain
        def emit_kv_mm(sc):
            ssl = slice(sc * SC, (sc + 1) * SC)
            pk = psN.tile([D, SC], f32, tag="b")
            pv = psN.tile([D, SC], f32, tag="b")
            pss = psN.tile([1, SC], f32, tag="b")
            for h2 in range(n_ht // 2):
                xt = x_p.tile([D, 2 * SC], f16, tag="x")
                nc.sync.dma_start(
                    xt[:],
                    xT_d[2 * h2 * D:(2 * h2 + 2) * D, ssl]
                    .rearrange("(j p) n -> p j n", j=2))
                for jj in range(2):
                    ht = 2 * h2 + jj
                    xv = xt[:, jj * SC:(jj + 1) * SC]
                    st, sp = ht == 0, ht == n_ht - 1
                    nc.tensor.matmul(pk[:], wkvq_sb[ht][:, 0:D], xv,
                                     start=st, stop=sp)
                    nc.tensor.matmul(pv[:], wkvq_sb[ht][:, D:2 * D], xv,
                                     start=st, stop=sp)
                    sqx = sq_p.tile([D, SC], f16, tag="sqx")
                    nc.vector.tensor_mul(sqx[:], xv, xv)
                    nc.tensor.matmul(pss[:], ones16[:], sqx[:], start=st, stop=sp)
            # ACT row chain for r + k-rms kicked off now (overlaps next chunk)
            tvar = row_p.tile([1, SC], f32, tag="row")
            nc.scalar.activation(tvar[:], pss[:], AF.Ln, scale=1.0 / H, bias=EPS)
            r_chunk = row_p.tile([1, SC], f32, tag="row")
            nc.scalar.activation(r_chunk[:], tvar[:], AF.Exp, scale=-0.5)
            # repack [1, SC] -> [D, n_st] per-partition layout via DRAM
            nc.sync.dma_start(r_dram[0:1, ssl], r_chunk[:])
            nc.sync.dma_start(
                r_pp[:, sc * n_st:(sc + 1) * n_st],
                r_dram[0:1, ssl].rearrange("o (j p) -> (o p) j", p=D))
            sqk = sq_p.tile([D, SC], f16, tag="sq")
            nc.scalar.activation(sqk[:], pk[:], AF.Square)
            return pk, pv, sqk

        def emit_kv_tail(sc, pk, pv, sqk):
            ssl = slice(sc * SC, (sc + 1) * SC)
            ckt = trig_p.tile([D, SC], f32, tag="ck")
            skt = trig_p.tile([D, SC], f32, tag="sk")
            nc.gpsimd.dma_start(ckt[:], ck_d[:, ssl])
            nc.gpsimd.dma_start(skt[:], sk_d[:, ssl])
            pss2 = psN.tile([1, SC], f32, tag="b")
            nc.tensor.matmul(pss2[:], ones16[:], sqk[:], start=True, stop=True)
            tvar = row_p.tile([1, SC], f32, tag="row")
            nc.scalar.activation(tvar[:], pss2[:], AF.Ln, scale=1.0 / D, bias=EPS)
            rq = row_p.tile([1, SC], f32r, tag="rowr")
            nc.scalar.activation(rq[:], tvar[:], AF.Exp, scale=-0.5)
            prb = psN.tile([D, SC], f32, tag="b")
            nc.tensor.matmul(prb[:], onesr[:], rq[:], start=True, stop=True)
            rb = rb_p.tile([D, SC], f32, tag="rb")
            nc.scalar.copy(rb[:], prb[:])
            rope_block(pk, ckt, skt, khatT[:, ssl], rb[:])
            # v: copy then transpose to [token, d], scaled by r
            vt = tmp_p.tile([D, SC], f32r, tag="vt")
            nc.vector.tensor_copy(vt[:], pv[:])
            for j in range(n_st):
                ptr = psN.tile([D, D], f32r, tag="b")
                nc.tensor.transpose(ptr[:], vt[:, j * D:(j + 1) * D], ident[:])
                kt = sc * n_st + j
                nc.scalar.activation(v_sb[:, kt * D:(kt + 1) * D],
                                     ptr[:].bitcast(f32), AF.Copy,
                                     scale=r_pp[:, kt:kt + 1])

        # ------------------------- QA phase ------------------------------
        proj_queue = []

        def drain_proj(keep):
            while len(proj_queue) > keep:
                emit_proj(proj_queue.pop(0))

        def emit_qa(sc):
            ssl = slice(sc * SC, (sc + 1) * SC)
            plan = chunk_plan[sc]
            # load whole x chunk resident (paired rows)
            xts = []
            for h2 in range(n_ht // 2):
                xt = xq_p.tile([D, 2 * SC], f16, tag="xq")
                nc.sync.dma_start(
                    xt[:],
                    xT_d[2 * h2 * D:(2 * h2 + 2) * D, ssl]
                    .rearrange("(j p) n -> p j n", j=2))
                xts.append(xt)
            if same_packs:
                cqt = trig_p.tile([D, SC], f32, tag="ck")
                sqt = trig_p.tile([D, SC], f32, tag="sk")
                nc.gpsimd.dma_start(cqt[:], ck_d[:, ssl])
                nc.gpsimd.dma_start(sqt[:], sk_d[:, ssl])
            else:
                cqt = trig_p.tile([D, SC], f32, tag="ck")
                sqt = trig_p.tile([D, SC], f32, tag="sk")
                nc.gpsimd.dma_start(cqt[:], cq_d[:, ssl])
                nc.gpsimd.dma_start(sqt[:], sq_d[:, ssl])

            pden3 = psN.tile([D, SC], f32, tag="b")  # head h uses row 32*h

            def emit_qproj(h):
                pq = psN.tile([D, SC], f32, tag="b", name=f"pq{h}")
                for ht in range(n_ht):
                    st, sp = ht == 0, ht == n_ht - 1
                    nc.tensor.matmul(pq[:], wkvq_sb[ht][:, (2 + h) * D:(3 + h) * D],
                                     xts[ht // 2][:, (ht % 2) * SC:(ht % 2 + 1) * SC],
                                     start=st, stop=sp)
                return pq

            def emit_qrope(pq):
                rbq = rms_scale(pq)
                qh = qh_p.tile([D, SC], f16, tag="qh")
                rope_block(pq, cqt, sqt, qh[:], rbq[:])
                return qh

            def emit_attention_head(h, qh):
                pattn = psN.tile([D, SC], f32, tag="b")
                pes = {}

                def emit_qk(i):
                    kt, a0, a1, mid = plan[i]
                    ps = psN.tile([D, SC], f32, tag="b")
                    nc.tensor.matmul(ps[:, a0:a1], khatT[:, kt * D:(kt + 1) * D],
                                     qh[:, a0:a1], start=True, stop=True)
                    if mid is not None:
                        mt = mask_p.tile([D, SC], f32, tag="m")
                        nc.sync.dma_start(mt[:], masks_d[mid, :, :])
                        nc.vector.tensor_add(ps[:, a0:a1], ps[:, a0:a1],
                                             mt[:, a0:a1])
                    pe = pexp_p.tile([D, SC], f16, tag="pe")
                    nc.scalar.activation(pe[:, a0:a1], ps[:, a0:a1], AF.Exp,
                                         scale=QKSCALE, bias=-1.0)
                    pes[i] = pe

                def emit_pv(i):
                    kt, a0, a1, mid = plan[i]
                    pe = pes.pop(i)
                    first, last = i == 0, i == len(plan) - 1
                    nc.tensor.matmul(pattn[:, a0:a1], v_sb[:, kt * D:(kt + 1) * D],
                                     pe[:, a0:a1], start=first, stop=last)
                    nc.tensor.matmul(pden3[32 * h:32 * h + 1, a0:a1], ones16[:], pe[:, a0:a1],
                                     start=first, stop=last)

                LAG = 3
                for i in range(len(plan)):
                    emit_qk(i)
                    if i >= LAG:
                        emit_pv(i - LAG)
                for i in range(max(0, len(plan) - LAG), len(plan)):
                    emit_pv(i)
                return pattn

            def emit_den(h, pattn):
                # normalize head h output by its softmax denominator
                at = at_p.tile([D, SC], f16, tag="at")
                if not plan:
                    nc.vector.memset(at[:], 0.0)
                else:
                    if uncov[sc] is not None:
                        nc.vector.tensor_add(pden3[32 * h:32 * h + 1, :],
                                             pden3[32 * h:32 * h + 1, :],
                                             uncov_sb[0:1, ssl])
                    rec = row_p.tile([1, SC], f32r, tag="rowr")
                    with nc.allow_low_precision(reason="1/den broadcast moving f32r"):
                        nc.vector.reciprocal(rec[:], pden3[32 * h:32 * h + 1, :])
                    prb = psN.tile([D, SC], f32, tag="b")
                    nc.tensor.matmul(prb[:], onesr[:], rec[:], start=True, stop=True)
                    rb2 = rb_p.tile([D, SC], f32, tag="rb")
                    nc.vector.tensor_copy(rb2[:], prb[:])
                    nc.vector.tensor_mul(at[:], pattn[:], rb2[:])
                nc.sync.dma_start(ag_in[sc][h * D:(h + 1) * D, :], at[:])

            # pipeline: PE queue stays fed with independent matmuls while the
            # ACT/DVE rms+rope chains run; head h's normalization is deferred
            # until after head h+1's attention matmuls
            pq0 = emit_qproj(0)
            pq1 = emit_qproj(1)
            qh0 = emit_qrope(pq0)
            pq2 = emit_qproj(2)
            qh1 = emit_qrope(pq1)
            pat0 = emit_attention_head(0, qh0)
            qh2 = emit_qrope(pq2)
            pat1 = emit_attention_head(1, qh1)
            emit_den(0, pat0)
            pat2 = emit_attention_head(2, qh2)
            emit_den(1, pat1)
            emit_den(2, pat2)
            nc.gpsimd.collective_compute(
                "AllGather", mybir.AluOpType.bypass,
                replica_groups=[list(range(NCORES))],
                ins=[ag_in[sc].ap()], outs=[ag_out[sc].ap()],
            )
            proj_queue.append(sc)

        # ---------------- main schedule ----------------
        load_wpt()
        pending_tail = None
        tails_done = -1
        early_emitted = False
        for sc in range(n_sc):
            mm = emit_kv_mm(sc)
            if pending_tail is not None:
                emit_kv_tail(*pending_tail)
                tails_done = pending_tail[0]
            pending_tail = (sc, *mm)
            if not early_emitted and tails_done >= maxk[early_sc]:
                emit_qa(early_sc)
                early_emitted = True
        emit_kv_tail(*pending_tail)
        if not early_emitted:
            emit_qa(early_sc)
        for sc in rest:
            emit_qa(sc)
            drain_proj(2)
        drain_proj(0)

    return nc


def build_and_run(x, cos, sin, pre_norm_w, q_norm_w, k_norm_w, Wq, Wk, Wv,
                  Wproj, q_ranges, k_ranges, cfg=None, trace=False,
                  trace_kwargs=None):
    from concourse.bass_utils import run_bass_kernel_spmd

    cfg = cfg or FULL_CFG
    per_core, spec = _host_prep(x, cos, sin, pre_norm_w, q_norm_w, k_norm_w,
                                Wq, Wk, Wv, Wproj, q_ranges, k_ranges, cfg)
    n_masks = per_core[0]["masks"].shape[0]
    same_packs = (np.array_equal(per_core[0]["cospack_q"], per_core[0]["cospack_k"])
                  and np.array_equal(per_core[0]["sinpack_q"], per_core[0]["sinpack_k"]))
    nc = _build_program(cfg, spec, n_masks, same_packs)
    _patch_bass(nc)

    in_maps = []
    for c in range(NCORES):
        m = dict(per_core[c])
        if any(u is not None for u in spec["uncov"]):
            S = cfg["S"]
            ua = np.zeros((1, S), np.float32)
            for sc, u in enumerate(spec["uncov"]):
                if u is not None:
                    ua[0, sc * SC:(sc + 1) * SC] = u
            m["uncov"] = ua
        in_maps.append(m)

    kw = {}
    if trace:
        kw = dict(trace=True, trace_kwargs=trace_kwargs or {})
    res = run_bass_kernel_spmd(nc, in_maps, core_ids=list(range(NCORES)), **kw)
    out = np.concatenate([res.results[c]["out"] for c in range(NCORES)], axis=0).T
    if not spec["all_covered"]:
        out = out * spec["covered"].T  # zero uncovered rows
    return out, res


def kernel(**inputs):
    out, _ = build_and_run(**inputs)
    return out
